# revision 40
# baseline (speedup 1.0000x reference)
"""Trainium2 Bass kernel for nn_Adjacency (dense_mlp).

Reference computation:
    pr = product @ w1[:S]                # [P, S]
    pe = person  @ w1[S:]                # [Q, S]
    h  = softplus(pr[:,None,:] + pe[None,:,:])   # [P, Q, S]
    m  = einsum('pqs,so->pq', h, w2)
    adj = leaky_relu(m, 0.1)
    out = adj[None] * x                  # [B, P, Q]

Sharding: P across 8 cores (128 rows each); person/w1/w2 replicated;
x / out sharded on dim 1. No collectives.

Per-core algorithm (all tiles [partition, free]):
  - pr_T/pe_T via TensorE matmuls (host passes pre-transposed operands,
    bf16); Enpr = exp(-pr_T) [s,p], Epe = exp(pe_T) [s,q] on ACT.
  - for p in 0..127: h'_p[s, q] = ln(Epe + Enpr[:, p]) — ONE ACT
    instruction per p, using the identity
       softplus(pr+pe) = ln(e^-pr + e^pe) + pr.
    The per-partition `bias` AP carries e^-pr (single aux AP — a second
    aux AP costs ~90ns/instruction); the dropped +pr term re-enters the
    reduction as a per-p constant c[p] = sum_s w2[s] pr[s,p] folded into
    the evacuation ops. The hardware has no softplus table; exp, ln and
    parametric_relu share one table set. This Ln stream runs at the ACT
    engine's back-to-back floor (1024 + 222 access cycles)/1.2GHz =
    1038 ns/instruction, ~133 us total — the kernel's critical path.
  - S-reduction on TensorE: m rows via M=128 matmuls whose lhsT has w2
    embedded in columns {j, j+32, j+64, j+96} and zeros elsewhere: 32
    accumulating matmuls per p-group build a PSUM tile whose four
    32-partition col-groups each hold the group's 32 adjacency rows
    (4x replicated). Replication makes the epilogue batch-packable.
  - per-group epilogue pipelined under the Ln stream: leaky-relu + c
    evacuation (DVE; ACT Prelu with bias=c for the last group), then TWO
    full-width bf16 multiplies per group against batch-packed x tiles
    (partition = 4 batches x 32 rows), DMA out in the packed layout
    (host unscrambles).
  - ~12 dummy matmuls at t=0 keep the PE HAM clock-gate warm so a cold
    (1.2 GHz) PE never paces the ACT stream; a dummy exp preloads the
    ACT table set under the weight DMAs.

Measured: ~154 us NEFF exec (all 8 cores within +-1.5 us), rel err ~6e-3
(bf16 h/x/adj rounding; gate is 2e-2).
"""

import numpy as np

P, Q, S, B = 1024, 1024, 128, 8
N_CORES = 8
PS = P // N_CORES  # 128 p rows per core
GROUPS = 4         # p-groups of 32 (PSUM col-groups)
GW = PS // GROUPS  # 32

_CACHE = {}


def _build_nc():
    import concourse.bass as bass
    import concourse.tile as tile
    from concourse import mybir
    from concourse.tile import add_dep_helper

    f32 = mybir.dt.float32
    bf16 = mybir.dt.bfloat16
    AF = mybir.ActivationFunctionType
    ALU = mybir.AluOpType

    nc = bass.Bass()

    # Weight blobs sized so the head-critical DMAs are small and each
    # matmul waits on ONE DMA semaphore (walrus allows a single sync wait
    # per instruction). wa1/wa2 feed the pe path, wb the pr path; the big
    # 4-column-embedded w2 (wc) is only needed ~40us in.
    QQ = Q // 4                       # 256-column person quarters
    WA1 = S + QQ                      # w1b | person_t[:, :256]
    WB = PS + S + 1                   # product_t | w1a | w2col
    wa1 = nc.declare_dram_parameter("wa1", [S, WA1], bf16, isOutput=False)
    wa2 = nc.declare_dram_parameter("wa2", [S, QQ], bf16, isOutput=False)
    wa3 = nc.declare_dram_parameter("wa3", [S, QQ], bf16, isOutput=False)
    wa4 = nc.declare_dram_parameter("wa4", [S, QQ], bf16, isOutput=False)
    wb = nc.declare_dram_parameter("wb", [S, WB], bf16, isOutput=False)
    wc = nc.declare_dram_parameter("wc", [S, GW, PS], bf16, isOutput=False)
    # x / out in epilogue-packed layout: [g, bb, 32*b' + k, q] <->
    # [b = 4*bb + b', p_local = 32*g + k, q]. Linear per (g, bb) tile so
    # every DMA is one big PDMA2D descriptor (scattered 3D APs cost ~5us
    # of SWDGE descriptor generation per 256KB).
    x_in = nc.declare_dram_parameter("x", [GROUPS, 2, PS, Q], bf16, isOutput=False)
    out_e = nc.declare_dram_parameter("out", [GROUPS, 2, PS, Q], bf16, isOutput=True)

    with tile.TileContext(nc) as tc:
        with (
            tc.tile_pool(name="const", bufs=1) as const,
            tc.tile_pool(name="xbuf", bufs=1) as xbuf,
            tc.tile_pool(name="hbuf", bufs=6) as hbuf,
            tc.tile_pool(name="pa", bufs=2, space="PSUM") as pa,
            tc.tile_pool(name="pm", bufs=2, space="PSUM") as pm,
        ):
            # Preload the exp/ln ACT table set while DMAs run: a dummy exp
            # on a memset tile has no input deps.
            scratch = const.tile([S, 1], f32)
            nc.vector.memset(scratch[:], 0.0)
            nc.scalar.activation(out=scratch[:], in_=scratch[:], func=AF.Exp)


            # ---- load weights (parallel queues for the head-critical pe path) ----
            wa1_sb = const.tile([S, WA1], bf16)
            wa2_sb = const.tile([S, QQ], bf16)
            wa3_sb = const.tile([S, QQ], bf16)
            wa4_sb = const.tile([S, QQ], bf16)
            wb_sb = const.tile([S, WB], bf16)
            w2e_sb = const.tile([S, GW, PS], bf16)
            nc.sync.dma_start(out=wa1_sb[:], in_=wa1[:])
            nc.sync.dma_start(out=wa2_sb[:], in_=wa2[:])
            nc.sync.dma_start(out=wa3_sb[:], in_=wa3[:])
            nc.sync.dma_start(out=wa4_sb[:], in_=wa4[:])
            nc.sync.dma_start(out=wb_sb[:], in_=wb[:])
            nc.sync.dma_start(out=w2e_sb[:], in_=wc[:])
            w1b_sb = wa1_sb[:, 0:S]
            pers_q = [
                wa1_sb[:, S : S + QQ],
                wa2_sb[:, 0:QQ],
                wa3_sb[:, 0:QQ],
                wa4_sb[:, 0:QQ],
            ]
            prod_sb = wb_sb[:, 0:PS]
            w1a_sb = wb_sb[:, PS : PS + S]
            w2c_sb = wb_sb[:, PS + S : PS + S + 1]

            # ---- pr_T / pe_T + exp ----
            # softplus(pr+pe) = ln(e^-pr + e^pe) + pr: the bias AP carries
            # e^-pr per partition; the +pr term folds into the reduction as
            # a per-p constant c[p] = sum_s w2[s] pr[s,p], applied at
            # evacuation time.
            enpr = const.tile([S, PS], f32)  # exp(-pr_T) [s, p]
            epe = const.tile([S, Q], f32)    # exp(pe_T) [s, q]

            for h in range(4):
                ps_pe = pa.tile([S, QQ], f32, tag="pe")
                nc.tensor.matmul(out=ps_pe[:], lhsT=w1b_sb, rhs=pers_q[h])
                nc.scalar.activation(
                    out=epe[:, h * QQ : (h + 1) * QQ], in_=ps_pe[:], func=AF.Exp
                )
            ps_pr = pa.tile([S, QQ], f32, tag="pe")
            nc.tensor.matmul(out=ps_pr[:, :PS], lhsT=w1a_sb, rhs=prod_sb)
            nc.scalar.activation(
                out=enpr[:], in_=ps_pr[:, :PS], func=AF.Exp, scale=-1.0
            )

            # c[p] replicated per col-group: crep[32c+j, g] = c[32g+j],
            # via 16 tiny M=32 matmuls (lhsT = pr_T slice, rhs = w2 column).
            pr_sb = const.tile([S, PS], bf16)
            nc.vector.tensor_copy(out=pr_sb[:], in_=ps_pr[:, :PS])
            crep_ps = pa.tile([PS, 4], f32, tag="c")
            for g in range(GROUPS):
                for c in range(4):
                    nc.tensor.matmul(
                        out=crep_ps[32 * c : 32 * c + 32, g : g + 1],
                        lhsT=pr_sb[:, GW * g : GW * (g + 1)],
                        rhs=w2c_sb,
                        tile_position=(0, 32 * c),
                    )
            crep = const.tile([PS, 4], f32)
            nc.vector.tensor_copy(out=crep[:], in_=crep_ps[:])

            # PE warmup: dummy matmuls emitted after the setup matmuls so
            # they keep the PE busy (HAM clock-gate at 2.4 GHz) across the
            # idle window before the main loop, without delaying the exp
            # chain. A cold (1.2 GHz) PE paces the ACT stream (~30us).
            wsrc = const.tile([S, QQ], bf16)
            nc.vector.memset(wsrc[:], 0.0)
            for _ in range(14):
                wtile = pa.tile([S, QQ], f32, tag="pe")
                nc.tensor.matmul(out=wtile[:], lhsT=wsrc[:, :S], rhs=wsrc[:])

            # Deferred loads issued from the gpsimd queue, gated on epe:
            # keep w2emb + x off the HBM while the head-critical DMAs run.
            gate = const.tile([S, 1], f32)
            g_ins = nc.gpsimd.tensor_copy(out=gate[:], in_=epe[:, 0:1])
            # x batch-packed: xp[g][bb] partition 32*b' + k  <->  x[4*bb+b', 32*g+k]
            xp = []
            for g in range(GROUPS):
                row = []
                for bb in range(2):
                    t = xbuf.tile([PS, Q], bf16, tag=f"xp{g}{bb}")
                    d = nc.gpsimd.dma_start(out=t[:], in_=x_in[g, bb])
                    add_dep_helper(d.ins, g_ins.ins, False, "x after epe gate")
                    row.append(t)
                xp.append(row)

            # ---- main loop with per-group pipelined epilogue ----
            adjr = const.tile([PS, Q], bf16)   # group adj rows, 4x replicated
            tmp = const.tile([PS, Q], bf16)
            for g in range(GROUPS):
                m_ps = pm.tile([PS, Q], f32)
                gsl = slice(GW * g, GW * (g + 1))
                for j in range(GW):
                    p = GW * g + j
                    h_t = hbuf.tile([S, Q], bf16, tag="h")
                    # h_p = ln(1 + Epe * Epr[:, p])  == softplus(pr_p + pe)
                    nc.scalar.activation(
                        out=h_t[:],
                        in_=epe[:],
                        func=AF.Ln,
                        bias=enpr[:, p : p + 1],
                    )
                    for hh in range(2):
                        qsl = slice(hh * (Q // 2), (hh + 1) * (Q // 2))
                        nc.tensor.matmul(
                            out=m_ps[:, qsl],
                            lhsT=w2e_sb[:, j, :],
                            rhs=h_t[:, qsl],
                            start=(j == 0),
                            stop=(j == GW - 1),
                        )
                # leaky relu evacuation: adjr = max(m, 0.1*m), full width
                # (all four col-groups at once). DVE while hidden under the
                # Ln stream; ACT Prelu (same table set) for the last group.
                if g < GROUPS - 1:
                    # a DVE op may read only one PSUM operand -> two steps;
                    # both fold in the +c correction
                    nc.vector.tensor_scalar(
                        tmp[:], m_ps[:], crep[:, g : g + 1], 0.1,
                        op0=ALU.add, op1=ALU.mult,
                    )
                    nc.vector.scalar_tensor_tensor(
                        out=adjr[:], in0=m_ps[:], scalar=crep[:, g : g + 1],
                        in1=tmp[:], op0=ALU.add, op1=ALU.max,
                    )
                else:
                    nc.scalar.activation(
                        out=adjr[:], in_=m_ps[:], func=AF.Prelu,
                        bias=crep[:, g : g + 1], alpha=0.1,
                    )
                # out = adjr * x, batch-packed: two full-width muls cover all
                # 8 batches for this group's rows.
                for bb in range(2):
                    op_t = xbuf.tile([PS, Q], bf16, tag=f"op{bb}")
                    nc.vector.tensor_mul(out=op_t[:], in0=xp[g][bb][:], in1=adjr[:])
                    nc.sync.dma_start(out=out_e[g, bb], in_=op_t[:])

    _fix_waits(nc)
    return nc


_ENGINE_SEM_PREFIX = {
    "EngineType.PE": "PE_",
    "EngineType.Activation": "Activation_",
    "EngineType.DVE": "DVE_",
    "EngineType.Pool": "Pool_",
    "EngineType.SP": "SP_sequencer_",
}


def _fix_waits(nc):
    """Make every instruction carry at most ONE semaphore wait (the TRN2
    ISA / neuronx-cc walrus limit).

    1. Strip waits on an instruction's own engine semaphore: engines
       execute strictly in order, so same-engine WAW/WAR waits (emitted by
       Tile's non-transitive vector clock) are always already satisfied.
    2. Strip same-queue ordering waits on DMAs (sem also in on_update):
       hardware DMA queues are FIFO and none of our DMAs have data deps on
       each other.
    3. Hoist any remaining extra waits onto same-engine NoOps inserted
       right before the instruction (waits execute sequentially on the
       sequencer).
    """
    from concourse import mybir

    for f in nc.m.functions:
        for bb in f.blocks:
            for ins in bb.instructions:
                si = ins.sync_info
                if si is None or not si.on_wait:
                    continue
                drop = set()
                pref = _ENGINE_SEM_PREFIX.get(str(getattr(ins, "engine", "")))
                if pref is not None:
                    drop.update(
                        w.ant_name
                        for w in si.on_wait
                        if (w.ant_name or "").startswith(pref)
                    )
                if str(ins.opcode) == "DMACopy":
                    upd = {u.ant_name for u in (si.on_update or [])}
                    drop.update(w.ant_name for w in si.on_wait if w.ant_name in upd)
                if drop:
                    kept = [w for w in si.on_wait if w.ant_name not in drop]
                    ins.sync_info = mybir.SyncInfo(
                        on_wait=kept, on_update=list(si.on_update or [])
                    )

    for f in nc.m.functions:
        for bb in f.blocks:
            out = []
            for ins in bb.instructions:
                si = ins.sync_info
                if si is not None and si.on_wait and len(si.on_wait) > 1:
                    waits = list(si.on_wait)
                    for k, w in enumerate(waits[:-1]):
                        nop = mybir.InstNoOp(name=f"{ins.name}-hw{k}", ins=[], outs=[])
                        nop.engine = ins.engine
                        nop.sync_info = mybir.SyncInfo(on_wait=[w], on_update=[])
                        out.append(nop)
                    ins.sync_info = mybir.SyncInfo(
                        on_wait=[waits[-1]], on_update=list(si.on_update or [])
                    )
                out.append(ins)
            bb.instructions = out


def _get_nc():
    if "nc" not in _CACHE:
        _CACHE["nc"] = _build_nc()
    return _CACHE["nc"]


def make_in_maps(x, product, person, w1, w2):
    import ml_dtypes

    bf16 = ml_dtypes.bfloat16
    x = np.asarray(x, dtype=np.float32)
    product = np.asarray(product, dtype=np.float32)
    person = np.asarray(person, dtype=np.float32)
    w1 = np.asarray(w1, dtype=np.float32)
    w2 = np.asarray(w2, dtype=np.float32)

    pers_t = np.ascontiguousarray(person.T)           # [S, Q]
    w1a = np.ascontiguousarray(w1[:S])                # [S, S]
    w1b = np.ascontiguousarray(w1[S:])                # [S, S]

    # 4-column-embedded w2: wc[k, j, m] = w2[k] if m % 32 == j else 0.
    # Accumulating over j fills each of the four 32-partition col-groups
    # with the group's 32 adjacency rows.
    wc = np.zeros((S, GW, PS), dtype=np.float32)
    jj = np.arange(GW)
    for c in range(4):
        wc[:, jj, 32 * c + jj] = w2[:, 0][:, None]
    wc = wc.astype(bf16)

    QQ = Q // 4
    wa1 = np.concatenate([w1b, pers_t[:, :QQ]], axis=1).astype(bf16)
    wa2 = pers_t[:, QQ : 2 * QQ].astype(bf16)
    wa3 = pers_t[:, 2 * QQ : 3 * QQ].astype(bf16)
    wa4 = pers_t[:, 3 * QQ :].astype(bf16)
    x_bf = x.astype(bf16)

    in_maps = []
    for i in range(N_CORES):
        sl = slice(PS * i, PS * (i + 1))
        wb = np.concatenate(
            [np.ascontiguousarray(product[sl].T), w1a, w2], axis=1
        ).astype(bf16)
        # pack x[b, p_local, q] -> [g, bb, 32*b' + k, q]
        xc = x_bf[:, sl, :].reshape(2, 4, GROUPS, GW, Q)      # [bb, b', g, k, q]
        xp = np.ascontiguousarray(xc.transpose(2, 0, 1, 3, 4)).reshape(
            GROUPS, 2, PS, Q
        )
        in_maps.append(
            {
                "wa1": np.ascontiguousarray(wa1),
                "wa2": np.ascontiguousarray(wa2),
                "wa3": np.ascontiguousarray(wa3),
                "wa4": np.ascontiguousarray(wa4),
                "wb": wb,
                "wc": wc,
                "x": xp,
            }
        )
    return in_maps


def run(x, product, person, w1, w2, trace=False, **kw):
    from concourse.bass_utils import run_bass_kernel_spmd

    nc = _get_nc()
    in_maps = make_in_maps(x, product, person, w1, w2)
    res = run_bass_kernel_spmd(
        nc, in_maps, core_ids=list(range(N_CORES)), trace=trace, **kw
    )
    outs = []
    for r in res.results:
        o = np.asarray(r["out"])                     # [g, bb, 32*b'+k, q] bf16
        o = o.reshape(GROUPS, 2, 4, GW, Q).transpose(1, 2, 0, 3, 4)
        outs.append(o.reshape(B, PS, Q).astype(np.float32))
    full = np.concatenate(outs, axis=1)
    return full, res


def kernel(x, product, person, w1, w2):
    full, _ = run(x, product, person, w1, w2, trace=False)
    return full
